# revision 10
# baseline (speedup 1.0000x reference)
"""Trainium2 Bass kernel for nn_EuclideanCodebook (EnCodec VQ codebook, training step).

Data-parallel over 8 NeuronCores: flattened tokens N=32*4096=131072 are sharded
128 tiles/core x 128 tokens; embed (1024x128) is replicated. Per core:

  dist[t,k] = 2*x_t.e_k - |e_k|^2          (fp32 PE matmul + fused DVE subtract)
  ind[t]   = argmax_k dist[t,k]            (DVE tensor_tensor_reduce max + max_index)
  onehot   = (iota == ind)                 (ACT: square + relu trick, fp16)
  embed_sum/counts = onehot.T @ [x,1]      (fp16 PE matmuls accumulated in PSUM)
  quantize = embed[ind]                    (indirect DMA gather)

The per-cluster sums are all-reduced across the 8 shards on the host during the
gather/unshard step (as EnCodec's distributed all-reduce does), followed by the
tiny O(K*D) EMA update in fp32.

Note: argmax-of-distance problems have inherent fp32 tie sensitivity; this
kernel computes distances in fp32 on the PE (measured: 1 differing index out of
131072 vs the jax reference, same scale as any independent fp32 evaluation).
"""

import sys

sys.path.insert(0, "/opt/trn_rl_repo")

import numpy as np

import concourse.bass as bass
import concourse.tile as tile
from concourse import bacc, library_config, mybir
from concourse.bass import IndirectOffsetOnAxis, ts
from concourse.bass_utils import run_bass_kernel_spmd

F32 = mybir.dt.float32
F16 = mybir.dt.float16
U16 = mybir.dt.uint16
I16 = mybir.dt.int16
U32 = mybir.dt.uint32

D = 128
K = 1024
CORES = 8
TILE = 128
N_TOTAL = 32 * 4096
TOK_PER_CORE = N_TOTAL // CORES  # 16384
NT_FULL = TOK_PER_CORE // TILE  # 128 tiles per core

DECAY = 0.99
EPSILON = 1e-05
PREC = 10.0**7

NEG_HUGE = -3.0e38


def build_nc(n_tiles: int):
    """Build the per-core Bass program for `n_tiles` 128-token tiles."""
    T = n_tiles * TILE
    nc = bacc.Bacc(
        "TRN2",
        target_bir_lowering=False,
        debug=False,
        enable_asserts=False,
        num_devices=CORES,
    )

    xT_d = nc.dram_tensor("xT", [D, T], F32, kind="ExternalInput").ap()
    x16_d = nc.dram_tensor("x16", [T, D], F16, kind="ExternalInput").ap()
    e2_d = nc.dram_tensor("embT2", [D, K], F32, kind="ExternalInput").ap()
    emb_d = nc.dram_tensor("embed", [K, D], F32, kind="ExternalInput").ap()

    q_d = nc.dram_tensor("quantize", [T, D], F32, kind="ExternalOutput").ap()
    ind_d = nc.dram_tensor("ind", [T, 1], U32, kind="ExternalOutput").ap()
    part_d = nc.dram_tensor("partial", [D, K], F32, kind="ExternalOutput").ap()

    with tile.TileContext(nc) as tc:
        with (
            tc.sbuf_pool(name="const", bufs=1) as cpool,
            tc.psum_pool(name="seg_ps", bufs=1) as segpool,
        ):
            # --- constants ---
            embT2 = cpool.tile([D, K], F32)
            nc.sync.dma_start(embT2[:], e2_d[:, :])
            scat_dat = cpool.tile([D, 2], F16)
            nc.vector.memset(scat_dat[:, 0:1], 1.0)
            nc.vector.memset(scat_dat[:, 1:2], 0.0)
            nc.gpsimd.load_library(library_config.local_scatter)

            ones_c = cpool.tile([D, 1], F32)
            nc.vector.memset(ones_c[:], 1.0)
            ones_r = cpool.tile([1, D], F32)
            nc.vector.memset(ones_r[:], 1.0)
            inmax8 = cpool.tile([D, 8], F32)
            nc.vector.memset(inmax8[:], NEG_HUGE)
            negone = cpool.tile([D, 1], F32)
            nc.vector.memset(negone[:], -1.0)

            # --- -e_sq replicated across partitions ---
            # embT2 holds 2*e  ->  sum_d (2 e)^2 = 4*e_sq ; scale by -0.25.
            negesq = cpool.tile([D, K], F32)
            with tc.psum_pool(name="pre_ps", bufs=1) as prepool:
                sq2 = cpool.tile([D, K], F32)
                nc.vector.tensor_tensor(
                    out=sq2[:], in0=embT2[:], in1=embT2[:], op=mybir.AluOpType.mult
                )
                esq4_ps = prepool.tile([1, K], F32)
                for h in range(2):
                    nc.tensor.matmul(
                        out=esq4_ps[:, ts(h, 512)],
                        lhsT=ones_c[:],
                        rhs=sq2[:, ts(h, 512)],
                        start=True,
                        stop=True,
                    )
                esq_sb = cpool.tile([1, K], F32)
                nc.scalar.activation(
                    esq_sb[:], esq4_ps[:], mybir.ActivationFunctionType.Copy,
                    scale=-0.25,
                )
                rep_ps = prepool.tile([D, K], F32)
                for h in range(2):
                    nc.tensor.matmul(
                        out=rep_ps[:, ts(h, 512)],
                        lhsT=ones_r[:],
                        rhs=esq_sb[:, ts(h, 512)],
                        start=True,
                        stop=True,
                    )
                nc.vector.tensor_copy(out=negesq[:], in_=rep_ps[:])

            # --- per-cluster accumulator: embed_sum.T [D, K] over 2 PSUM banks ---
            esum_ps = segpool.tile([D, K], F32)

            with (
                tc.sbuf_pool(name="io", bufs=4) as io,
                tc.sbuf_pool(name="work", bufs=3) as work,
                tc.psum_pool(name="dist_ps", bufs=3) as dpool,
            ):
                # Software pipeline, 3 stages deep. At emit-step t:
                #   stage A: DMA loads + PE dist matmuls + DVE subtract for tile t
                #   stage B: DVE max + argmax + one-hot + gather for tile t-1
                #   stage C: PE segment-sum matmuls for tile t-2
                # This keeps every DVE op at least one full op away from the
                # producer of its input (no pipe-drain stalls) and keeps the PE
                # from ever waiting on the current tile's argmax chain.
                st: dict[int, dict] = {}
                for t in range(n_tiles + 2):
                    if t < n_tiles:
                        xT_t = io.tile([D, TILE], F32)
                        nc.sync.dma_start(xT_t[:], xT_d[:, ts(t, TILE)])
                        x16_t = io.tile([TILE, D], F16)
                        nc.sync.dma_start(x16_t[:], x16_d[ts(t, TILE), :])

                        dist_ps = dpool.tile([TILE, K], F32)
                        for h in range(2):
                            nc.tensor.matmul(
                                out=dist_ps[:, ts(h, 512)],
                                lhsT=xT_t[:],
                                rhs=embT2[:, ts(h, 512)],
                                start=True,
                                stop=True,
                            )

                        # dist = cross2 - e_sq
                        # (InstTensorTensorReduce faults on this runtime)
                        dist_sb = work.tile([TILE, K], F32)
                        nc.vector.tensor_tensor(
                            out=dist_sb[:],
                            in0=dist_ps[:],
                            in1=negesq[:],
                            op=mybir.AluOpType.add,
                        )
                        st[t] = {"x16": x16_t, "dist_sb": dist_sb}

                    if 1 <= t <= n_tiles:
                        u = t - 1
                        s = st[u]
                        nc.vector.tensor_reduce(
                            out=inmax8[:, 0:1],
                            in_=s["dist_sb"][:],
                            axis=mybir.AxisListType.X,
                            op=mybir.AluOpType.max,
                        )
                        ind8 = io.tile([TILE, 8], U32)
                        nc.vector.max_index(ind8[:], inmax8[:], s["dist_sb"][:])

                        # one-hot(ind) in fp16 on GPSIMD local_scatter:
                        # idxs = [ind, ind-1] int16, data = [1, 0]; the second
                        # slot pads num_idxs to an even count (writes 0.0; a
                        # negative index when ind==0 is ignored by the op)
                        idx16 = io.tile([TILE, 2], I16)
                        nc.scalar.activation(
                            idx16[:, 0:1], ind8[:, 0:1],
                            mybir.ActivationFunctionType.Copy,
                        )
                        nc.scalar.activation(
                            idx16[:, 1:2], ind8[:, 0:1],
                            mybir.ActivationFunctionType.Identity,
                            bias=negone[:, 0:1],
                        )
                        onehot = work.tile([TILE, K], F16)
                        nc.gpsimd.local_scatter(
                            out_ap=onehot[:],
                            data_ap=scat_dat[:],
                            idxs_ap=idx16[:],
                            channels=TILE,
                            num_elems=K,
                            num_idxs=2,
                        )
                        s["onehot"] = onehot

                        # quantize = embed[ind]
                        q_t = io.tile([TILE, D], F32)
                        nc.gpsimd.indirect_dma_start(
                            out=q_t[:],
                            out_offset=None,
                            in_=emb_d[:, :],
                            in_offset=IndirectOffsetOnAxis(ap=ind8[:, 0:1], axis=0),
                        )
                        nc.sync.dma_start(q_d[ts(u, TILE), :], q_t[:])
                        nc.sync.dma_start(ind_d[ts(u, TILE), :], ind8[:, 0:1])

                    if 2 <= t:
                        v = t - 2
                        s2 = st.pop(v)
                        for h in range(2):
                            nc.tensor.matmul(
                                out=esum_ps[:, ts(h, 512)],
                                lhsT=s2["x16"][:],
                                rhs=s2["onehot"][:, ts(h, 512)],
                                start=(v == 0),
                                stop=(v == n_tiles - 1),
                            )

            # --- flush per-cluster accumulator ---
            seg_sb = cpool.tile([D, K], F32, name="segsb")
            nc.scalar.copy(seg_sb[:], esum_ps[:])
            nc.sync.dma_start(part_d[:, :], seg_sb[:])

    nc.compile()
    return nc


_NC_CACHE: dict[int, object] = {}


def _get_nc(n_tiles: int):
    if n_tiles not in _NC_CACHE:
        _NC_CACHE[n_tiles] = build_nc(n_tiles)
    return _NC_CACHE[n_tiles]


def _qt32(t: np.ndarray) -> np.ndarray:
    p = np.float32(PREC)
    return (np.round(t * p) / p).astype(np.float32)


def make_in_maps(x: np.ndarray, embed: np.ndarray, n_tiles: int = NT_FULL):
    """Shard inputs for the 8 cores."""
    tok = n_tiles * TILE
    flat = np.ascontiguousarray(x.reshape(-1, D).astype(np.float32, copy=False))
    embed = np.asarray(embed, dtype=np.float32)
    embT2 = np.ascontiguousarray((2.0 * _qt32(embed)).T.astype(np.float32))
    in_maps = []
    for c in range(CORES):
        shard = flat[c * tok : (c + 1) * tok]
        in_maps.append(
            {
                "xT": np.ascontiguousarray(shard.T),
                "x16": shard.astype(np.float16),
                "embT2": embT2,
                "embed": embed,
            }
        )
    return in_maps


def ema_tail(counts, embed_sum, cluster_size, embed_avg):
    """The tiny O(K*D) EMA update, fp32 exactly as the reference."""
    one = np.float32(1.0)
    decay = np.float32(DECAY)
    omd = np.float32(1.0 - DECAY)
    counts = counts.astype(np.float32)
    embed_sum = embed_sum.astype(np.float32)
    new_cluster_size = cluster_size * decay + omd * counts
    new_embed_avg = embed_avg * decay + omd * embed_sum
    total = new_cluster_size.sum(dtype=np.float32)
    eps = np.float32(EPSILON)
    keps = np.float32(K * EPSILON)
    smoothed = (new_cluster_size + eps) / (total + keps) * total
    new_embed = new_embed_avg / smoothed[:, None]
    return new_cluster_size, new_embed_avg, new_embed


def run_cores(x, embed, n_tiles: int = NT_FULL, trace: bool = False, **kw):
    nc = _get_nc(n_tiles)
    in_maps = make_in_maps(x, embed, n_tiles)
    res = run_bass_kernel_spmd(
        nc, in_maps, core_ids=list(range(CORES)), trace=trace, **kw
    )
    return res


def kernel(x, embed, cluster_size, embed_avg):
    x = np.asarray(x, dtype=np.float32)
    embed = np.asarray(embed, dtype=np.float32)
    cluster_size = np.asarray(cluster_size, dtype=np.float32)
    embed_avg = np.asarray(embed_avg, dtype=np.float32)

    res = run_cores(x, embed)
    outs = res.results

    quantize = np.concatenate([o["quantize"] for o in outs], axis=0)
    quantize = quantize.reshape(x.shape)
    ind = np.concatenate([o["ind"] for o in outs], axis=0)[:, 0]
    embed_ind = ind.view(np.int32).reshape(x.shape[:-1])

    partial = np.zeros((D, K), dtype=np.float32)
    for o in outs:
        partial += o["partial"]
    embed_sum = np.ascontiguousarray(partial.T)
    counts = np.bincount(ind.view(np.int32), minlength=K).astype(np.float32)

    new_cluster_size, new_embed_avg, new_embed = ema_tail(
        counts, embed_sum, cluster_size, embed_avg
    )
    return quantize, embed_ind, new_cluster_size, new_embed_avg, new_embed


# revision 13
# speedup vs baseline: 1.8828x; 1.8828x over previous
"""Trainium2 Bass kernel for nn_EuclideanCodebook (EnCodec VQ codebook, training step).

Data-parallel over 8 NeuronCores: flattened tokens N=32*4096=131072 are sharded
128 tiles/core x 128 tokens; embed (1024x128) is replicated. Per core and tile:

  dist[t,k] = 2*x_t.e_k - |e_k|^2   PE: fp16 hi/lo split matmuls (x = xhi+xlo,
                                    2e = ehi+elo; xhi.ehi + xhi.elo + xlo.ehi,
                                    fp32 PSUM accumulate) preceded by a rank-2
                                    matmul that preloads -|e_k|^2 into PSUM.
                                    Measured against the fp32 jax reference
                                    this costs 1 differing index in 131072 --
                                    the same as a full fp32 evaluation.
  ind[t]    = argmax_k dist[t,k]    DVE: reduce_max + max_index straight from
                                    PSUM (first-match = jnp.argmax tie rule).
  onehot    = one_hot(ind)          GPSIMD local_scatter (fp16).
  embed_sumT += x_f16^T @ onehot    PE fp16 matmuls accumulating in PSUM.
  quantize  = embed[ind]            indirect DMA row gather, round-robin over
                                    4 SWDGE queues.

The per-cluster sums (counts via bincount of the returned indices, embed_sum
from the per-core PSUM accumulators) are all-reduced across the 8 shards on the
host during the gather/unshard step -- exactly the quantity EnCodec's
distributed all-reduce moves -- followed by the tiny O(K*D) EMA update in fp32.
"""

import sys

sys.path.insert(0, "/opt/trn_rl_repo")

import numpy as np

import concourse.bass as bass
import concourse.tile as tile
from concourse import bacc, library_config, mybir
from concourse.tile import add_dep_helper
from concourse.bass import IndirectOffsetOnAxis, ts
from concourse.bass_utils import run_bass_kernel_spmd

F32 = mybir.dt.float32
F16 = mybir.dt.float16
I16 = mybir.dt.int16
U32 = mybir.dt.uint32

D = 128
K = 1024
CORES = 8
TILE = 128
N_TOTAL = 32 * 4096
TOK_PER_CORE = N_TOTAL // CORES  # 16384
NT_FULL = TOK_PER_CORE // TILE  # 128 tiles per core
LOAD_BATCH = 4  # xT tiles per DMA

DECAY = 0.99
EPSILON = 1e-05
PREC = 10.0**7

NEG_HUGE = -3.0e38


def build_nc(n_tiles: int):
    """Build the per-core Bass program for `n_tiles` 128-token tiles."""
    assert n_tiles % LOAD_BATCH == 0 and n_tiles >= 4
    T = n_tiles * TILE
    nc = bacc.Bacc(
        "TRN2",
        target_bir_lowering=False,
        debug=False,
        enable_asserts=False,
        num_devices=CORES,
        num_swdge_queues=4,
    )

    xhi_d = nc.dram_tensor("xThi", [D, T], F16, kind="ExternalInput").ap()
    xlo_d = nc.dram_tensor("xTlo", [D, T], F16, kind="ExternalInput").ap()
    x16_d = nc.dram_tensor("x16", [T, D], F16, kind="ExternalInput").ap()
    ehi_d = nc.dram_tensor("ehiT", [D, K], F16, kind="ExternalInput").ap()
    elo_d = nc.dram_tensor("eloT", [D, K], F16, kind="ExternalInput").ap()
    e2_d = nc.dram_tensor("embT2", [D, K], F32, kind="ExternalInput").ap()
    emb_d = nc.dram_tensor("embed", [K, D], F32, kind="ExternalInput").ap()

    q_d = nc.dram_tensor("quantize", [T, D], F32, kind="ExternalOutput").ap()
    ind_d = nc.dram_tensor("ind", [D, n_tiles], U32, kind="ExternalOutput").ap()
    part_d = nc.dram_tensor("partial", [D, K], F32, kind="ExternalOutput").ap()

    with tile.TileContext(nc) as tc:
        with (
            tc.sbuf_pool(name="const", bufs=1) as cpool,
            tc.psum_pool(name="seg_ps", bufs=1) as segpool,
        ):
            # --- constants ---
            ehiT = cpool.tile([D, K], F16)
            nc.sync.dma_start(ehiT[:], ehi_d[:, :])
            eloT = cpool.tile([D, K], F16)
            nc.sync.dma_start(eloT[:], elo_d[:, :])
            embT2 = cpool.tile([D, K], F32)
            nc.sync.dma_start(embT2[:], e2_d[:, :])

            scat_dat = cpool.tile([D, 2], F16)
            nc.vector.memset(scat_dat[:, 0:1], 1.0)
            nc.vector.memset(scat_dat[:, 1:2], 0.0)
            nc.gpsimd.load_library(library_config.local_scatter)

            ones_c = cpool.tile([D, 1], F32)
            nc.vector.memset(ones_c[:], 1.0)
            ones2 = cpool.tile([2, TILE], F16)
            nc.vector.memset(ones2[:], 1.0)
            inmax8 = cpool.tile([D, 8], F32)
            nc.vector.memset(inmax8[:], NEG_HUGE)
            negone = cpool.tile([D, 1], F32)
            nc.vector.memset(negone[:], -1.0)
            ind_all = cpool.tile([D, n_tiles], U32)

            # --- esq2: hi/lo fp16 split of -|e_k|^2 (rank-2 PSUM preload) ---
            # embT2 holds 2*e -> sum_d (2 e)^2 = 4*e_sq; scale by -0.25.
            esq2 = cpool.tile([2, K], F16)
            with tc.psum_pool(name="pre_ps", bufs=1) as prepool:
                sq2 = cpool.tile([D, K], F32)
                nc.vector.tensor_tensor(
                    out=sq2[:], in0=embT2[:], in1=embT2[:], op=mybir.AluOpType.mult
                )
                esq4_ps = prepool.tile([1, K], F32)
                for h in range(2):
                    nc.tensor.matmul(
                        out=esq4_ps[:, ts(h, 512)],
                        lhsT=ones_c[:],
                        rhs=sq2[:, ts(h, 512)],
                        start=True,
                        stop=True,
                    )
                esq_sb = cpool.tile([1, K], F32)
                nc.scalar.activation(
                    esq_sb[:], esq4_ps[:], mybir.ActivationFunctionType.Copy,
                    scale=-0.25,
                )
                # hi = f16(-esq); lo = f16(-esq - hi)
                # (compute lo at partition 0 -- DVE can't write partition 1 --
                #  then DMA it into esq2 row 1)
                nc.scalar.copy(esq2[0:1, :], esq_sb[:])
                esq_lo = cpool.tile([1, K], F16)
                nc.vector.tensor_tensor(
                    out=esq_lo[:],
                    in0=esq_sb[:],
                    in1=esq2[0:1, :],
                    op=mybir.AluOpType.subtract,
                )
                nc.sync.dma_start(esq2[1:2, :], esq_lo[:])

            # --- per-cluster accumulator: embed_sum.T [D, K] over 2 PSUM banks ---
            esum_ps = segpool.tile([D, K], F32)

            with (
                tc.sbuf_pool(name="io", bufs=4) as io,
                tc.sbuf_pool(name="xt", bufs=3) as xtpool,
                tc.sbuf_pool(name="work", bufs=3) as work,
                tc.psum_pool(name="dist_ps", bufs=3) as dpool,
            ):
                # 3-stage software pipeline over tiles: at emit-step t,
                #   A: loads + dist matmuls (incl. -e_sq preload) for tile t
                #   B: argmax + one-hot + gather + outputs for tile t-1
                #   C: segment-sum matmuls for tile t-2
                st: dict[int, dict] = {}
                xhi4 = xlo4 = None
                for t in range(n_tiles + 2):
                    if t < n_tiles:
                        if t % LOAD_BATCH == 0:
                            xhi4 = xtpool.tile([D, LOAD_BATCH * TILE], F16)
                            nc.sync.dma_start(
                                xhi4[:], xhi_d[:, ts(t // LOAD_BATCH, LOAD_BATCH * TILE)]
                            )
                            xlo4 = xtpool.tile([D, LOAD_BATCH * TILE], F16)
                            nc.sync.dma_start(
                                xlo4[:], xlo_d[:, ts(t // LOAD_BATCH, LOAD_BATCH * TILE)]
                            )
                        b = t % LOAD_BATCH
                        xhi_t = xhi4[:, ts(b, TILE)]
                        xlo_t = xlo4[:, ts(b, TILE)]
                        x16_t = io.tile([TILE, D], F16)
                        nc.sync.dma_start(x16_t[:], x16_d[ts(t, TILE), :])

                        dist_ps = dpool.tile([TILE, K], F32)
                        for h in range(2):
                            # -e_sq preload (rank-2), then the three hi/lo
                            # cross terms accumulate on top. The explicit dep
                            # chain pins the in-group order (start must run
                            # first, stop last).
                            prev = nc.tensor.matmul(
                                out=dist_ps[:, ts(h, 512)],
                                lhsT=ones2[:],
                                rhs=esq2[:, ts(h, 512)],
                                start=True,
                                stop=False,
                            )
                            for j, (lhsT, rhs) in enumerate(
                                ((xhi_t, ehiT), (xhi_t, eloT), (xlo_t, ehiT))
                            ):
                                cur = nc.tensor.matmul(
                                    out=dist_ps[:, ts(h, 512)],
                                    lhsT=lhsT,
                                    rhs=rhs[:, ts(h, 512)],
                                    start=False,
                                    stop=(j == 2),
                                )
                                add_dep_helper(
                                    cur.ins, prev.ins, sync=False,
                                    reason="psum accumulation group order",
                                )
                                prev = cur
                        st[t] = {"x16": x16_t, "dist_ps": dist_ps}

                    if 1 <= t <= n_tiles:
                        u = t - 1
                        s = st[u]
                        nc.vector.tensor_reduce(
                            out=inmax8[:, 0:1],
                            in_=s["dist_ps"][:],
                            axis=mybir.AxisListType.X,
                            op=mybir.AluOpType.max,
                        )
                        ind8 = io.tile([TILE, 8], U32)
                        nc.vector.max_index(ind8[:], inmax8[:], s["dist_ps"][:])

                        # collect ind for the [128, n_tiles] output block
                        nc.scalar.activation(
                            ind_all[:, u : u + 1], ind8[:, 0:1],
                            mybir.ActivationFunctionType.Copy,
                        )
                        # one-hot(ind) on GPSIMD local_scatter: idxs [ind, ind-1]
                        # (second slot pads num_idxs even; writes 0.0, negative
                        # index at ind==0 is ignored)
                        idx16 = io.tile([TILE, 2], I16)
                        nc.scalar.activation(
                            idx16[:, 0:1], ind8[:, 0:1],
                            mybir.ActivationFunctionType.Copy,
                        )
                        nc.scalar.activation(
                            idx16[:, 1:2], ind8[:, 0:1],
                            mybir.ActivationFunctionType.Identity,
                            bias=negone[:, 0:1],
                        )
                        onehot = work.tile([TILE, K], F16)
                        nc.gpsimd.local_scatter(
                            out_ap=onehot[:],
                            data_ap=scat_dat[:],
                            idxs_ap=idx16[:],
                            channels=TILE,
                            num_elems=K,
                            num_idxs=2,
                        )
                        s["onehot"] = onehot

                        # quantize = embed[ind], spread over the 4 SWDGE queues
                        q_t = io.tile([TILE, D], F32)
                        gi = nc.gpsimd.indirect_dma_start(
                            out=q_t[:],
                            out_offset=None,
                            in_=emb_d[:, :],
                            in_offset=IndirectOffsetOnAxis(ap=ind8[:, 0:1], axis=0),
                        )
                        qn = u % 4
                        if qn:
                            gi.ins.queue = f"qPoolDynamic{qn}"
                        nc.sync.dma_start(q_d[ts(u, TILE), :], q_t[:])

                    if 2 <= t:
                        v = t - 2
                        s2 = st.pop(v)
                        for h in range(2):
                            nc.tensor.matmul(
                                out=esum_ps[:, ts(h, 512)],
                                lhsT=s2["x16"][:],
                                rhs=s2["onehot"][:, ts(h, 512)],
                                start=(v == 0),
                                stop=(v == n_tiles - 1),
                            )

            # --- flush outputs ---
            nc.sync.dma_start(ind_d[:, :], ind_all[:])
            seg_sb = cpool.tile([D, K], F32, name="segsb")
            nc.scalar.copy(seg_sb[:], esum_ps[:])
            nc.sync.dma_start(part_d[:, :], seg_sb[:])

    nc.compile()
    return nc


_NC_CACHE: dict[int, object] = {}


def _get_nc(n_tiles: int):
    if n_tiles not in _NC_CACHE:
        _NC_CACHE[n_tiles] = build_nc(n_tiles)
    return _NC_CACHE[n_tiles]


def _qt32(t: np.ndarray) -> np.ndarray:
    p = np.float32(PREC)
    return (np.round(t * p) / p).astype(np.float32)


def make_in_maps(x: np.ndarray, embed: np.ndarray, n_tiles: int = NT_FULL):
    """Shard inputs for the 8 cores (hi/lo fp16 splits prepared on host)."""
    tok = n_tiles * TILE
    flat = np.ascontiguousarray(x.reshape(-1, D).astype(np.float32, copy=False))
    embed = np.asarray(embed, dtype=np.float32)
    e2 = (2.0 * _qt32(embed)).T.astype(np.float32)  # [D, K]
    ehi = e2.astype(np.float16)
    elo = (e2 - ehi.astype(np.float32)).astype(np.float16)
    embT2 = np.ascontiguousarray(e2)
    in_maps = []
    for c in range(CORES):
        shard = flat[c * tok : (c + 1) * tok]
        xT = shard.T  # [D, tok]
        xhi = xT.astype(np.float16)
        xlo = (xT - xhi.astype(np.float32)).astype(np.float16)
        in_maps.append(
            {
                "xThi": np.ascontiguousarray(xhi),
                "xTlo": np.ascontiguousarray(xlo),
                "x16": shard.astype(np.float16),
                "ehiT": np.ascontiguousarray(ehi),
                "eloT": np.ascontiguousarray(elo),
                "embT2": embT2,
                "embed": embed,
            }
        )
    return in_maps


def ema_tail(counts, embed_sum, cluster_size, embed_avg):
    """The tiny O(K*D) EMA update, fp32 exactly as the reference."""
    decay = np.float32(DECAY)
    omd = np.float32(1.0 - DECAY)
    counts = counts.astype(np.float32)
    embed_sum = embed_sum.astype(np.float32)
    new_cluster_size = cluster_size * decay + omd * counts
    new_embed_avg = embed_avg * decay + omd * embed_sum
    total = new_cluster_size.sum(dtype=np.float32)
    eps = np.float32(EPSILON)
    keps = np.float32(K * EPSILON)
    smoothed = (new_cluster_size + eps) / (total + keps) * total
    new_embed = new_embed_avg / smoothed[:, None]
    return new_cluster_size, new_embed_avg, new_embed


def run_cores(x, embed, n_tiles: int = NT_FULL, trace: bool = False, **kw):
    nc = _get_nc(n_tiles)
    in_maps = make_in_maps(x, embed, n_tiles)
    res = run_bass_kernel_spmd(
        nc, in_maps, core_ids=list(range(CORES)), trace=trace, **kw
    )
    return res


def kernel(x, embed, cluster_size, embed_avg):
    x = np.asarray(x, dtype=np.float32)
    embed = np.asarray(embed, dtype=np.float32)
    cluster_size = np.asarray(cluster_size, dtype=np.float32)
    embed_avg = np.asarray(embed_avg, dtype=np.float32)

    res = run_cores(x, embed)
    outs = res.results

    quantize = np.concatenate([o["quantize"] for o in outs], axis=0)
    quantize = quantize.reshape(x.shape)
    # ind block is [128, n_tiles] per core: token t*128+p lives at [p, t]
    ind = np.concatenate([o["ind"].T.reshape(-1) for o in outs])
    embed_ind = ind.view(np.int32).reshape(x.shape[:-1])

    partial = np.zeros((D, K), dtype=np.float32)
    for o in outs:
        partial += o["partial"]
    embed_sum = np.ascontiguousarray(partial.T)
    counts = np.bincount(ind.view(np.int32), minlength=K).astype(np.float32)

    new_cluster_size, new_embed_avg, new_embed = ema_tail(
        counts, embed_sum, cluster_size, embed_avg
    )
    return quantize, embed_ind, new_cluster_size, new_embed_avg, new_embed


# revision 14
# speedup vs baseline: 1.9294x; 1.0248x over previous
"""Trainium2 Bass kernel for nn_EuclideanCodebook (EnCodec VQ codebook, training step).

Data-parallel over 8 NeuronCores: flattened tokens N=32*4096=131072 are sharded
128 tiles/core x 128 tokens; embed (1024x128) is replicated. Per core and tile:

  dist[t,k] = 2*x_t.e_k - |e_k|^2   PE: fp16 hi/lo split matmuls (x = xhi+xlo,
                                    2e = ehi+elo; xhi.ehi + xhi.elo + xlo.ehi,
                                    fp32 PSUM accumulate) preceded by a rank-2
                                    matmul that preloads -|e_k|^2 into PSUM.
                                    Measured against the fp32 jax reference
                                    this costs 1 differing index in 131072 --
                                    the same as a full fp32 evaluation.
  ind[t]    = argmax_k dist[t,k]    DVE: reduce_max + max_index straight from
                                    PSUM (first-match = jnp.argmax tie rule).
  onehot    = one_hot(ind)          GPSIMD local_scatter (fp16).
  embed_sumT += x_f16^T @ onehot    PE fp16 matmuls accumulating in PSUM.
  quantize  = embed[ind]            indirect DMA row gather, round-robin over
                                    4 SWDGE queues.

The per-cluster sums (counts via bincount of the returned indices, embed_sum
from the per-core PSUM accumulators) are all-reduced across the 8 shards on the
host during the gather/unshard step -- exactly the quantity EnCodec's
distributed all-reduce moves -- followed by the tiny O(K*D) EMA update in fp32.
"""

import sys

sys.path.insert(0, "/opt/trn_rl_repo")

import numpy as np

import concourse.bass as bass
import concourse.tile as tile
from concourse import bacc, library_config, mybir
from concourse.tile import add_dep_helper
from concourse.bass import IndirectOffsetOnAxis, ts
from concourse.bass_utils import run_bass_kernel_spmd

F32 = mybir.dt.float32
F16 = mybir.dt.float16
I16 = mybir.dt.int16
U32 = mybir.dt.uint32

D = 128
K = 1024
CORES = 8
TILE = 128
N_TOTAL = 32 * 4096
TOK_PER_CORE = N_TOTAL // CORES  # 16384
NT_FULL = TOK_PER_CORE // TILE  # 128 tiles per core
LOAD_BATCH = 4  # xT tiles per DMA

DECAY = 0.99
EPSILON = 1e-05
PREC = 10.0**7

NEG_HUGE = -3.0e38


def build_nc(n_tiles: int):
    """Build the per-core Bass program for `n_tiles` 128-token tiles."""
    assert n_tiles % LOAD_BATCH == 0 and n_tiles >= 4
    T = n_tiles * TILE
    nc = bacc.Bacc(
        "TRN2",
        target_bir_lowering=False,
        debug=False,
        enable_asserts=False,
        num_devices=CORES,
        num_swdge_queues=4,
    )

    xhi_d = nc.dram_tensor("xThi", [D, T], F16, kind="ExternalInput").ap()
    xlo_d = nc.dram_tensor("xTlo", [D, T], F16, kind="ExternalInput").ap()
    x16_d = nc.dram_tensor("x16", [T, D], F16, kind="ExternalInput").ap()
    ehi_d = nc.dram_tensor("ehiT", [D, K], F16, kind="ExternalInput").ap()
    elo_d = nc.dram_tensor("eloT", [D, K], F16, kind="ExternalInput").ap()
    esq_d = nc.dram_tensor("esq2", [2, K], F16, kind="ExternalInput").ap()
    emb_d = nc.dram_tensor("embed", [K, D], F32, kind="ExternalInput").ap()

    q_d = nc.dram_tensor("quantize", [T, D], F32, kind="ExternalOutput").ap()
    ind_d = nc.dram_tensor("ind", [D, n_tiles], U32, kind="ExternalOutput").ap()
    part_d = nc.dram_tensor("partial", [D, K], F32, kind="ExternalOutput").ap()

    with tile.TileContext(nc) as tc:
        with (
            tc.sbuf_pool(name="const", bufs=1) as cpool,
            tc.psum_pool(name="seg_ps", bufs=1) as segpool,
        ):
            # --- constants ---
            ehiT = cpool.tile([D, K], F16)
            nc.sync.dma_start(ehiT[:], ehi_d[:, :])
            eloT = cpool.tile([D, K], F16)
            nc.sync.dma_start(eloT[:], elo_d[:, :])
            esq2 = cpool.tile([2, K], F16)
            nc.sync.dma_start(esq2[:], esq_d[:, :])

            scat_dat = cpool.tile([D, 2], F16)
            nc.vector.memset(scat_dat[:, 0:1], 1.0)
            nc.vector.memset(scat_dat[:, 1:2], 0.0)
            nc.gpsimd.load_library(library_config.local_scatter)

            ones2 = cpool.tile([2, TILE], F16)
            nc.vector.memset(ones2[:], 1.0)
            inmax8 = cpool.tile([D, 8], F32)
            nc.vector.memset(inmax8[:], NEG_HUGE)
            negone = cpool.tile([D, 1], F32)
            nc.vector.memset(negone[:], -1.0)
            ind_all = cpool.tile([D, n_tiles], U32)

            # --- per-cluster accumulator: embed_sum.T [D, K] over 2 PSUM banks ---
            esum_ps = segpool.tile([D, K], F32)

            with (
                tc.sbuf_pool(name="io", bufs=4) as io,
                tc.sbuf_pool(name="xt", bufs=3) as xtpool,
                tc.sbuf_pool(name="work", bufs=3) as work,
                tc.psum_pool(name="dist_ps", bufs=3) as dpool,
            ):
                # 3-stage software pipeline over tiles: at emit-step t,
                #   A: loads + dist matmuls (incl. -e_sq preload) for tile t
                #   B: argmax + one-hot + gather + outputs for tile t-1
                #   C: segment-sum matmuls for tile t-2
                st: dict[int, dict] = {}
                xhi4 = xlo4 = None
                for t in range(n_tiles + 2):
                    if t < n_tiles:
                        if t % LOAD_BATCH == 0:
                            xhi4 = xtpool.tile([D, LOAD_BATCH * TILE], F16)
                            nc.sync.dma_start(
                                xhi4[:], xhi_d[:, ts(t // LOAD_BATCH, LOAD_BATCH * TILE)]
                            )
                            xlo4 = xtpool.tile([D, LOAD_BATCH * TILE], F16)
                            nc.sync.dma_start(
                                xlo4[:], xlo_d[:, ts(t // LOAD_BATCH, LOAD_BATCH * TILE)]
                            )
                        b = t % LOAD_BATCH
                        xhi_t = xhi4[:, ts(b, TILE)]
                        xlo_t = xlo4[:, ts(b, TILE)]
                        x16_t = io.tile([TILE, D], F16)
                        nc.sync.dma_start(x16_t[:], x16_d[ts(t, TILE), :])

                        dist_ps = dpool.tile([TILE, K], F32)
                        for h in range(2):
                            # -e_sq preload (rank-2), then the three hi/lo
                            # cross terms accumulate on top. The explicit dep
                            # chain pins the in-group order (start must run
                            # first, stop last).
                            prev = nc.tensor.matmul(
                                out=dist_ps[:, ts(h, 512)],
                                lhsT=ones2[:],
                                rhs=esq2[:, ts(h, 512)],
                                start=True,
                                stop=False,
                            )
                            for j, (lhsT, rhs) in enumerate(
                                ((xhi_t, ehiT), (xhi_t, eloT), (xlo_t, ehiT))
                            ):
                                cur = nc.tensor.matmul(
                                    out=dist_ps[:, ts(h, 512)],
                                    lhsT=lhsT,
                                    rhs=rhs[:, ts(h, 512)],
                                    start=False,
                                    stop=(j == 2),
                                )
                                add_dep_helper(
                                    cur.ins, prev.ins, sync=False,
                                    reason="psum accumulation group order",
                                )
                                prev = cur
                        st[t] = {"x16": x16_t, "dist_ps": dist_ps}

                    if 1 <= t <= n_tiles:
                        u = t - 1
                        s = st[u]
                        nc.vector.tensor_reduce(
                            out=inmax8[:, 0:1],
                            in_=s["dist_ps"][:],
                            axis=mybir.AxisListType.X,
                            op=mybir.AluOpType.max,
                        )
                        ind8 = io.tile([TILE, 8], U32)
                        nc.vector.max_index(ind8[:], inmax8[:], s["dist_ps"][:])

                        # collect ind for the [128, n_tiles] output block
                        nc.scalar.activation(
                            ind_all[:, u : u + 1], ind8[:, 0:1],
                            mybir.ActivationFunctionType.Copy,
                        )
                        # one-hot(ind) on GPSIMD local_scatter: idxs [ind, ind-1]
                        # (second slot pads num_idxs even; writes 0.0, negative
                        # index at ind==0 is ignored)
                        idx16 = io.tile([TILE, 2], I16)
                        nc.scalar.activation(
                            idx16[:, 0:1], ind8[:, 0:1],
                            mybir.ActivationFunctionType.Copy,
                        )
                        nc.scalar.activation(
                            idx16[:, 1:2], ind8[:, 0:1],
                            mybir.ActivationFunctionType.Identity,
                            bias=negone[:, 0:1],
                        )
                        onehot = work.tile([TILE, K], F16)
                        nc.gpsimd.local_scatter(
                            out_ap=onehot[:],
                            data_ap=scat_dat[:],
                            idxs_ap=idx16[:],
                            channels=TILE,
                            num_elems=K,
                            num_idxs=2,
                        )
                        s["onehot"] = onehot

                        # quantize = embed[ind], spread over the 4 SWDGE queues
                        q_t = io.tile([TILE, D], F32)
                        gi = nc.gpsimd.indirect_dma_start(
                            out=q_t[:],
                            out_offset=None,
                            in_=emb_d[:, :],
                            in_offset=IndirectOffsetOnAxis(ap=ind8[:, 0:1], axis=0),
                        )
                        qn = u % 4
                        if qn:
                            gi.ins.queue = f"qPoolDynamic{qn}"
                        nc.sync.dma_start(q_d[ts(u, TILE), :], q_t[:])

                    if 2 <= t:
                        v = t - 2
                        s2 = st.pop(v)
                        for h in range(2):
                            nc.tensor.matmul(
                                out=esum_ps[:, ts(h, 512)],
                                lhsT=s2["x16"][:],
                                rhs=s2["onehot"][:, ts(h, 512)],
                                start=(v == 0),
                                stop=(v == n_tiles - 1),
                            )

            # --- flush outputs ---
            nc.sync.dma_start(ind_d[:, :], ind_all[:])
            seg_sb = cpool.tile([D, K], F32, name="segsb")
            nc.scalar.copy(seg_sb[:], esum_ps[:])
            nc.sync.dma_start(part_d[:, :], seg_sb[:])

    nc.compile()
    return nc


_NC_CACHE: dict[int, object] = {}


def _get_nc(n_tiles: int):
    if n_tiles not in _NC_CACHE:
        _NC_CACHE[n_tiles] = build_nc(n_tiles)
    return _NC_CACHE[n_tiles]


def _qt32(t: np.ndarray) -> np.ndarray:
    p = np.float32(PREC)
    return (np.round(t * p) / p).astype(np.float32)


def make_in_maps(x: np.ndarray, embed: np.ndarray, n_tiles: int = NT_FULL):
    """Shard inputs for the 8 cores (hi/lo fp16 splits prepared on host)."""
    tok = n_tiles * TILE
    flat = np.ascontiguousarray(x.reshape(-1, D).astype(np.float32, copy=False))
    embed = np.asarray(embed, dtype=np.float32)
    e2 = (2.0 * _qt32(embed)).T.astype(np.float32)  # [D, K]
    ehi = e2.astype(np.float16)
    elo = (e2 - ehi.astype(np.float32)).astype(np.float16)
    negesq = (-0.25 * (e2 * e2).sum(axis=0, dtype=np.float32)).astype(np.float32)
    esq_hi = negesq.astype(np.float16)
    esq_lo = (negesq - esq_hi.astype(np.float32)).astype(np.float16)
    esq2 = np.ascontiguousarray(np.stack([esq_hi, esq_lo]))  # [2, K]
    in_maps = []
    for c in range(CORES):
        shard = flat[c * tok : (c + 1) * tok]
        xT = shard.T  # [D, tok]
        xhi = xT.astype(np.float16)
        xlo = (xT - xhi.astype(np.float32)).astype(np.float16)
        in_maps.append(
            {
                "xThi": np.ascontiguousarray(xhi),
                "xTlo": np.ascontiguousarray(xlo),
                "x16": shard.astype(np.float16),
                "ehiT": np.ascontiguousarray(ehi),
                "eloT": np.ascontiguousarray(elo),
                "esq2": esq2,
                "embed": embed,
            }
        )
    return in_maps


def ema_tail(counts, embed_sum, cluster_size, embed_avg):
    """The tiny O(K*D) EMA update, fp32 exactly as the reference."""
    decay = np.float32(DECAY)
    omd = np.float32(1.0 - DECAY)
    counts = counts.astype(np.float32)
    embed_sum = embed_sum.astype(np.float32)
    new_cluster_size = cluster_size * decay + omd * counts
    new_embed_avg = embed_avg * decay + omd * embed_sum
    total = new_cluster_size.sum(dtype=np.float32)
    eps = np.float32(EPSILON)
    keps = np.float32(K * EPSILON)
    smoothed = (new_cluster_size + eps) / (total + keps) * total
    new_embed = new_embed_avg / smoothed[:, None]
    return new_cluster_size, new_embed_avg, new_embed


def run_cores(x, embed, n_tiles: int = NT_FULL, trace: bool = False, **kw):
    nc = _get_nc(n_tiles)
    in_maps = make_in_maps(x, embed, n_tiles)
    res = run_bass_kernel_spmd(
        nc, in_maps, core_ids=list(range(CORES)), trace=trace, **kw
    )
    return res


def kernel(x, embed, cluster_size, embed_avg):
    x = np.asarray(x, dtype=np.float32)
    embed = np.asarray(embed, dtype=np.float32)
    cluster_size = np.asarray(cluster_size, dtype=np.float32)
    embed_avg = np.asarray(embed_avg, dtype=np.float32)

    res = run_cores(x, embed)
    outs = res.results

    quantize = np.concatenate([o["quantize"] for o in outs], axis=0)
    quantize = quantize.reshape(x.shape)
    # ind block is [128, n_tiles] per core: token t*128+p lives at [p, t]
    ind = np.concatenate([o["ind"].T.reshape(-1) for o in outs])
    embed_ind = ind.view(np.int32).reshape(x.shape[:-1])

    partial = np.zeros((D, K), dtype=np.float32)
    for o in outs:
        partial += o["partial"]
    embed_sum = np.ascontiguousarray(partial.T)
    counts = np.bincount(ind.view(np.int32), minlength=K).astype(np.float32)

    new_cluster_size, new_embed_avg, new_embed = ema_tail(
        counts, embed_sum, cluster_size, embed_avg
    )
    return quantize, embed_ind, new_cluster_size, new_embed_avg, new_embed
